# revision 1
# baseline (speedup 1.0000x reference)
"""Causal multi-head attention on 8 Trainium2 NeuronCores.

Problem: nn_Attention_46643344835180
  x: [8, 1024, 768], 12 heads x 64 dh, causal softmax attention + output proj.

Sharding: data-parallel over batch (8 batch elements -> 8 cores, no collectives).

Per-core dataflow (batch element b):
  xT = x_b.T                       via PE transposes                  [768, 1024]
  QT = Wq_cat.T @ xT  (+bq)        heads stacked on partitions        [768, 1024]
  KT = Wk_cat.T @ xT  (+bk)                                           [768, 1024]
  V  = x_b @ Wv_cat   (+bv)        + interleaved ones column          [1024, 12*65]
  per head h, query-chunk qc (512):
    S^T[k,q] = KT_h.T @ QT_h          keys on partitions
    P^T = exp(S^T / 8)                ScalarE, batched over 2 key-blocks
    causal: one wide-mask multiply on the partial columns
    z^T[65,512] += [V_h | 1].T @ P^T  row 64 accumulates the denominator
    ZT_h = z^T[0:64] * approx(1/z^T[64])   (denom staged to row 0 ->
           reciprocal_approx_fast -> gpsimd partition_broadcast -> multiply)
  out = ZT.T @ Wo_cat (+bo)                                           [1024, 768]

Dtype config CFG = (bf_qk, bf_vproj, bf_pv, bf_o) picks bf16 vs f32r per stage.
"""

import sys

sys.path.insert(0, "/opt/trn_rl_repo")

import ml_dtypes
import numpy as np

import concourse.bass as bass
import concourse.mybir as mybir
import concourse.tile as tile
from concourse import bacc
from concourse.bass_utils import run_bass_kernel_spmd
from concourse.masks import make_identity

F32 = mybir.dt.float32
F32R = mybir.dt.float32r
BF16 = mybir.dt.bfloat16
AF = mybir.ActivationFunctionType

SEQ = 1024
DM = 768
NH = 12
DH = 64
BATCH = 8
NQT = SEQ // 128  # 8 seq tiles of 128
NDT = DM // 128  # 6 d_model tiles
QC = 512  # query chunk (moving dim)
NQC = SEQ // QC  # 2

# (bf_qk, bf_vproj, bf_pv, bf_o)
CFG = (False, False, False, False)


def _npdt(dt):
    return ml_dtypes.bfloat16 if dt == BF16 else np.float32


def build(with_bq, with_bk, with_bv, with_bo, cfg=CFG, debug_taps=False):
    bf_qk, bf_vproj, bf_pv, bf_o = cfg
    DT_QK = BF16 if bf_qk else F32R  # wq/wk, QT/KT, scores matmul
    DT_VP = BF16 if bf_vproj else F32R  # wv + V-projection compute
    DT_PV = BF16 if bf_pv else F32R  # V storage, ones, P^T, PV matmul
    DT_O = BF16 if bf_o else F32R  # ZT, wo, output matmul
    DT_MASK = BF16 if bf_pv else F32
    need_xtr = (not bf_qk) or (not bf_vproj)
    need_xtb = bf_qk or bf_vproj

    nc = bacc.Bacc("TRN2", target_bir_lowering=False, debug=False)

    x = nc.dram_tensor("x", [SEQ, DM], F32, kind="ExternalInput")
    wq = nc.dram_tensor("wq", [DM, DM], DT_QK, kind="ExternalInput")
    wk = nc.dram_tensor("wk", [DM, DM], DT_QK, kind="ExternalInput")
    wv = nc.dram_tensor("wv", [DM, DM], DT_VP, kind="ExternalInput")
    wo = nc.dram_tensor("wo", [DM, DM], DT_O, kind="ExternalInput")
    wmask = nc.dram_tensor("wmask", [128, 640], DT_MASK, kind="ExternalInput")
    identin = nc.dram_tensor("identin", [128, 128], F32, kind="ExternalInput")
    onesc = nc.dram_tensor("onesc", [128, NH], DT_PV, kind="ExternalInput")
    bq = bk = bv = bo = None
    if with_bq:
        bq = nc.dram_tensor("bq", [128, NDT], F32, kind="ExternalInput")
    if with_bk:
        bk = nc.dram_tensor("bk", [128, NDT], F32, kind="ExternalInput")
    if with_bv:
        bv = nc.dram_tensor("bv", [1, DM], F32, kind="ExternalInput")
    if with_bo:
        bo = nc.dram_tensor("bo", [1, DM], F32, kind="ExternalInput")
    out = nc.dram_tensor("out", [SEQ, DM], F32, kind="ExternalOutput")
    taps = {}
    if debug_taps:
        for nm in ("xT_d", "QT_d", "KT_d", "ZT_d"):
            taps[nm] = nc.dram_tensor(nm, [DM, SEQ], F32, kind="ExternalOutput")
        taps["V_d"] = nc.dram_tensor(
            "V_d", [SEQ, NH * (DH + 1)], F32, kind="ExternalOutput"
        )

    with tile.TileContext(nc) as tc:
        with (
            tc.tile_pool(name="persist", bufs=1) as persist,
            tc.tile_pool(name="xn", bufs=3) as xn_pool,
            tc.tile_pool(name="wstream", bufs=6) as w_pool,
            tc.tile_pool(name="wqk", bufs=14) as wqk_pool,
            tc.tile_pool(name="pt", bufs=4) as pt_pool,
            tc.tile_pool(name="small", bufs=2) as small,
            tc.tile_pool(name="outst", bufs=2) as out_pool,
            tc.tile_pool(name="ps_st", bufs=2, space="PSUM") as ps_st,
            tc.tile_pool(name="ps_z", bufs=3, space="PSUM") as ps_z,
            tc.tile_pool(name="ps_mm", bufs=1, space="PSUM") as ps_mm,
        ):
            # ---- x loads first (longest startup chain) ----
            xn = []
            for s in range(NQT):
                t = xn_pool.tile([128, DM], F32, tag="xn", name="xn")
                for piece in range(3):
                    lo, hi = piece * 256, (piece + 1) * 256
                    nc.sync.dma_start(
                        out=t[:, lo:hi], in_=x[s * 128 : (s + 1) * 128, lo:hi]
                    )
                xn.append(t)

            # ---- constants ----
            ident = persist.tile([128, 128], F32, tag="ident", name="ident")
            nc.sync.dma_start(out=ident, in_=identin[:, :])
            # HAM warmup: ~4us of dummy matmuls while the x DMAs land, so the
            # transposes/projections start at 2.4GHz instead of the cold 1.2GHz
            warm_ps = ps_mm.tile([128, 128], F32, tag="proj", name="warm", padded_shape=[128, QC])
            for _ in range(20):
                nc.tensor.matmul(warm_ps, lhsT=ident, rhs=ident, start=True, stop=True)
            wm_t = persist.tile([128, 640], DT_MASK, tag="wmask", name="wmask")
            nc.sync.dma_start(out=wm_t, in_=wmask[:, :])

            bias_tiles = {}
            if with_bq:
                t = persist.tile([128, NDT], F32, tag="bq", name="bq")
                nc.sync.dma_start(out=t, in_=bq[:, :])
                bias_tiles["bq"] = t
            if with_bk:
                t = persist.tile([128, NDT], F32, tag="bk", name="bk")
                nc.sync.dma_start(out=t, in_=bk[:, :])
                bias_tiles["bk"] = t
            if with_bv:
                t = persist.tile([128, DM], F32, tag="bv", name="bv")
                nc.sync.dma_start(out=t, in_=bv[0:1, :].to_broadcast((128, DM)))
                bias_tiles["bv"] = t
            if with_bo:
                t = persist.tile([128, DM], F32, tag="bo", name="bo")
                nc.sync.dma_start(out=t, in_=bo[0:1, :].to_broadcast((128, DM)))
                bias_tiles["bo"] = t

            # ---- persistent activations ----
            xTr = xTb = None
            if need_xtr:
                xTr = [
                    persist.tile([128, SEQ], F32R, tag=f"xTr{d}", name=f"xTr{d}")
                    for d in range(NDT)
                ]
            if need_xtb:
                xTb = [
                    persist.tile([128, SEQ], BF16, tag=f"xTb{d}", name=f"xTb{d}")
                    for d in range(NDT)
                ]
            xT_qk = xTb if bf_qk else xTr
            xT_vp = xTb if bf_vproj else xTr
            QT = [
                persist.tile([128, SEQ], DT_QK, tag=f"QT{d}", name=f"QT{d}")
                for d in range(NDT)
            ]
            KT = [
                persist.tile([128, SEQ], DT_QK, tag=f"KT{d}", name=f"KT{d}")
                for d in range(NDT)
            ]
            V = [
                persist.tile(
                    [128, NH * (DH + 1)], DT_PV, tag=f"V{s}", name=f"V{s}"
                )
                for s in range(NQT)
            ]
            for s in range(NQT):
                vv = V[s].rearrange("p (h e) -> p h e", e=DH + 1)
                nc.sync.dma_start(
                    out=vv[:, :, DH : DH + 1],
                    in_=onesc[:, :].rearrange("p (h o) -> p h o", o=1),
                )
            ZT = [
                persist.tile([128, SEQ], DT_O, tag=f"ZT{d}", name=f"ZT{d}")
                for d in range(NDT)
            ]

            # ---- phase A: transpose x to xT ----
            for s in range(NQT):
                for d in range(NDT):
                    pst = ps_st.tile(
                        [128, 128], F32, tag="st", name="tp", padded_shape=[128, 2 * QC]
                    )
                    nc.tensor.transpose(pst, xn[s][:, d * 128 : (d + 1) * 128], ident)
                    if need_xtr:
                        nc.vector.tensor_copy(xTr[d][:, s * 128 : (s + 1) * 128], pst)
                    if need_xtb:
                        nc.vector.tensor_copy(xTb[d][:, s * 128 : (s + 1) * 128], pst)

            def qk_load(hp):
                tiles = []
                for wsrc in (wq, wk):
                    wts = []
                    for d in range(NDT):
                        t = wqk_pool.tile([128, 128], DT_QK, tag="wqk", name="wqk")
                        nc.sync.dma_start(
                            out=t,
                            in_=wsrc[
                                d * 128 : (d + 1) * 128, hp * 128 : (hp + 1) * 128
                            ],
                        )
                        wts.append(t)
                    tiles.append(wts)
                return tiles

            # ---- phase B ----
            NVC = 2
            VC = DM // NVC  # 384
            wt = []
            for d in range(NDT):
                t = w_pool.tile([128, DM], DT_VP, tag="w", name="w")
                nc.sync.dma_start(out=t, in_=wv[d * 128 : (d + 1) * 128, :])
                wt.append(t)
            def qk_proj(hp, tiles):
                # project QT/KT tile hp (heads 2hp, 2hp+1) with streamed weights
                for wts, (dst, bkey) in zip(tiles, ((QT, "bq"), (KT, "bk"))):
                    for c in range(NQC):
                        acc = ps_mm.tile([128, QC], F32, tag="proj", name="proj")
                        for d in range(NDT):
                            nc.tensor.matmul(
                                acc,
                                lhsT=wts[d],
                                rhs=xT_qk[d][:, c * QC : (c + 1) * QC],
                                start=(d == 0),
                                stop=(d == NDT - 1),
                            )
                        o = dst[hp][:, c * QC : (c + 1) * QC]
                        if bkey in bias_tiles:
                            nc.vector.tensor_scalar_add(
                                o, acc, bias_tiles[bkey][:, hp : hp + 1]
                            )
                        else:
                            nc.scalar.activation(o, acc, AF.Copy)

            def v_proj(s, pool, tag):
                for c in range(NVC):
                    acc = pool.tile(
                        [128, VC], F32, tag=tag, name="vacc",
                        padded_shape=[128, 2 * QC] if tag == "st" else [128, QC],
                    )
                    for d in range(NDT):
                        nc.tensor.matmul(
                            acc,
                            lhsT=xT_vp[d][:, s * 128 : (s + 1) * 128],
                            rhs=wt[d][:, c * VC : (c + 1) * VC],
                            start=(d == 0),
                            stop=(d == NDT - 1),
                        )
                    nh2 = VC // DH  # heads per chunk (6)
                    o = V[s].rearrange("p (h e) -> p h e", e=DH + 1)[
                        :, c * nh2 : (c + 1) * nh2, 0:DH
                    ]
                    if "bv" in bias_tiles:
                        nc.vector.tensor_add(
                            o,
                            acc.rearrange("p (h e) -> p h e", e=DH),
                            bias_tiles["bv"][:, c * VC : (c + 1) * VC].rearrange(
                                "p (h e) -> p h e", e=DH
                            ),
                        )
                    else:
                        nc.scalar.activation(
                            o, acc.rearrange("p (h e) -> p h e", e=DH), AF.Copy
                        )

            qk_loads = [qk_load(0), qk_load(1)]
            qk_proj(0, qk_loads[0])
            for s in range(NQT):
                v_proj(s, ps_st, "st")

            # ---- phase C: attention, qc-major (QK proj + O-proj interleaved) ----
            def attn_unit(hp, c):
                zps = {}
                for px in (0, 64):  # head A in partitions 0:64, B in 64:128
                    zps[px] = ps_z.tile([128, QC], F32, tag="z", name="z")
                nkb = 4 * (c + 1)  # causal: key blocks 0..nkb-1
                for g in range(0, nkb, 2):  # groups of 2 key-blocks
                    gsz = min(2, nkb - g)
                    # columns [0:doff) of a diagonal block are fully causal-masked:
                    # skip them in scores and PV (ragged-N); stale st/pt contents
                    # in the skipped columns are never read downstream.
                    doffs = [max(0, (g + j) * 128 - c * QC) for j in range(gsz)]
                    sts = {}
                    for px in (0, 64):
                        sts[px] = ps_st.tile(
                            [128, gsz * QC], F32, tag="st", name="st"
                        )
                    for j in range(gsz):
                        kb = g + j
                        off = doffs[j]
                        for px in (0, 64):  # adjacent pair -> row-group packed
                            nc.tensor.matmul(
                                sts[px][:, j * QC + off : (j + 1) * QC],
                                lhsT=KT[hp][px : px + 64, kb * 128 : (kb + 1) * 128],
                                rhs=QT[hp][px : px + 64, c * QC + off : (c + 1) * QC],
                                start=True,
                                stop=True,
                            )
                    pts = {}
                    for px in (0, 64):
                        pt = pt_pool.tile([128, 2 * QC], DT_PV, tag="pt", name="pt")
                        # single exp over the whole group; columns skipped by the
                        # ragged matmuls hold stale-but-finite psum, never read.
                        nc.scalar.activation(
                            pt[:, : gsz * QC], sts[px], AF.Exp, scale=0.125
                        )
                        pts[px] = pt
                    for j in range(gsz):
                        kb = g + j
                        doff = kb * 128 - c * QC
                        off = doffs[j]
                        for px in (0, 64):
                            pt = pts[px]
                            if 0 <= doff < QC:  # diagonal block: fixed 128-wide triangle
                                blk = pt[:, j * QC + doff : j * QC + doff + 128]
                                nc.vector.tensor_mul(blk, blk, wm_t[:, 512:640])
                            h = 2 * hp + (1 if px else 0)
                            nc.tensor.matmul(
                                zps[px][0 : DH + 1, off:QC],
                                lhsT=V[kb][:, h * (DH + 1) : (h + 1) * (DH + 1)],
                                rhs=pt[:, j * QC + off : (j + 1) * QC],
                                start=(kb == 0),
                                stop=(kb == nkb - 1),
                            )
                for px in (0, 64):
                    dstage = small.tile([128, QC], F32, tag="dstage", name="dstage")
                    nc.vector.tensor_copy(dstage[0:1, :], zps[px][DH : DH + 1, :])
                    recip = small.tile([128, QC], F32, tag="recip", name="recip")
                    nc.vector.reciprocal_approx_fast(recip, dstage)
                    bcast = small.tile([64, QC], F32, tag="bcast", name="bcast")
                    nc.gpsimd.partition_broadcast(bcast, recip[0:1, :])
                    nc.vector.tensor_mul(
                        ZT[hp][px : px + 64, c * QC : (c + 1) * QC],
                        zps[px][0:64, :],
                        bcast,
                    )

            wo_tiles = []

            def o_proj(s_range, pool):
                for s in s_range:
                    ot = out_pool.tile([128, DM], F32, tag="ostage", name="ostage")
                    for c in range(NVC):
                        acc = pool.tile(
                            [128, VC],
                            F32,
                            tag="z" if pool is ps_z else "proj",
                            name="oacc",
                            padded_shape=[128, QC],
                        )
                        for d in range(NDT):
                            nc.tensor.matmul(
                                acc,
                                lhsT=ZT[d][:, s * 128 : (s + 1) * 128],
                                rhs=wo_tiles[d][:, c * VC : (c + 1) * VC],
                                start=(d == 0),
                                stop=(d == NDT - 1),
                            )
                        o = ot[:, c * VC : (c + 1) * VC]
                        if "bo" in bias_tiles:
                            nc.vector.tensor_add(
                                o, acc, bias_tiles["bo"][:, c * VC : (c + 1) * VC]
                            )
                        else:
                            nc.vector.tensor_copy(o, acc)
                    nc.sync.dma_start(out=out[s * 128 : (s + 1) * 128, :], in_=ot)

            qk_tiles = {0: qk_loads[0], 1: qk_loads[1]}
            for hp in range(NH // 2):
                if hp + 2 < NH // 2:
                    qk_tiles[hp + 2] = qk_load(hp + 2)
                if hp + 1 < NH // 2:
                    qk_proj(hp + 1, qk_tiles[hp + 1])
                if hp == 4:  # prefetch O-proj weights late in the qc=0 sweep
                    for d in range(NDT):
                        t = w_pool.tile([128, DM], DT_O, tag="w", name="w")
                        nc.sync.dma_start(out=t, in_=wo[d * 128 : (d + 1) * 128, :])
                        wo_tiles.append(t)
                attn_unit(hp, 0)
            # first half of the output projection (queries 0..511) as filler
            o_proj(range(0, NQT // 2), ps_mm)
            for hp in range(NH // 2):
                attn_unit(hp, 1)

            # ---- phase D: output projection, second half ----
            o_proj(range(NQT // 2, NQT), ps_z)

            if debug_taps:
                for nm, tiles in (("QT_d", QT), ("KT_d", KT), ("ZT_d", ZT)):
                    for d in range(NDT):
                        nc.sync.dma_start(
                            out=taps[nm][d * 128 : (d + 1) * 128, :],
                            in_=tiles[d][:, :].bitcast(F32),
                        )
                xt_tap = xTr if need_xtr else xTb
                for d in range(NDT):
                    nc.sync.dma_start(
                        out=taps["xT_d"][d * 128 : (d + 1) * 128, :],
                        in_=xt_tap[d][:, :].bitcast(F32),
                    )
                for s in range(NQT):
                    nc.sync.dma_start(
                        out=taps["V_d"][s * 128 : (s + 1) * 128, :],
                        in_=V[s][:, :].bitcast(F32),
                    )

    nc.compile()
    return nc


_CACHE = {}


def _get_nc(key, cfg):
    k = (key, cfg)
    if k not in _CACHE:
        _CACHE[k] = build(*key, cfg=cfg)
    return _CACHE[k]


def _prep(inputs, cfg=CFG):
    bf_qk, bf_vproj, bf_pv, bf_o = cfg
    x = np.ascontiguousarray(np.asarray(inputs["normalized_resid_pre"], np.float32))
    dt_qk = _npdt(BF16 if bf_qk else F32R)
    dt_vp = _npdt(BF16 if bf_vproj else F32R)
    dt_pv = _npdt(BF16 if bf_pv else F32R)
    dt_o = _npdt(BF16 if bf_o else F32R)
    dt_mask = _npdt(BF16 if bf_pv else F32)
    wq = np.ascontiguousarray(
        np.asarray(inputs["W_Q"], np.float32).transpose(1, 0, 2).reshape(DM, DM)
    ).astype(dt_qk)
    wk = np.ascontiguousarray(
        np.asarray(inputs["W_K"], np.float32).transpose(1, 0, 2).reshape(DM, DM)
    ).astype(dt_qk)
    wv = np.ascontiguousarray(
        np.asarray(inputs["W_V"], np.float32).transpose(1, 0, 2).reshape(DM, DM)
    ).astype(dt_vp)
    wo = np.ascontiguousarray(
        np.asarray(inputs["W_O"], np.float32).reshape(DM, DM)
    ).astype(dt_o)
    bq = np.asarray(inputs["b_Q"], np.float32).reshape(NDT, 128).T
    bk = np.asarray(inputs["b_K"], np.float32).reshape(NDT, 128).T
    bv = np.asarray(inputs["b_V"], np.float32).reshape(1, DM)
    bo = np.asarray(inputs["b_O"], np.float32).reshape(1, DM)
    jj, uu = np.meshgrid(np.arange(128), np.arange(640), indexing="ij")
    wmask = (uu - 512 >= jj).astype(dt_mask)
    onesc = np.ones((128, NH), dt_pv)
    key = (
        bool(np.any(bq)),
        bool(np.any(bk)),
        bool(np.any(bv)),
        bool(np.any(bo)),
    )
    common = {
        "wq": wq, "wk": wk, "wv": wv, "wo": wo, "wmask": wmask, "onesc": onesc,
        "identin": np.eye(128, dtype=np.float32),
    }
    if key[0]:
        common["bq"] = np.ascontiguousarray(bq)
    if key[1]:
        common["bk"] = np.ascontiguousarray(bk)
    if key[2]:
        common["bv"] = np.ascontiguousarray(bv)
    if key[3]:
        common["bo"] = np.ascontiguousarray(bo)
    in_maps = [dict(common, x=np.ascontiguousarray(x[b])) for b in range(BATCH)]
    return key, in_maps


def run(inputs, trace=False, cfg=CFG, **kw):
    key, in_maps = _prep(inputs, cfg)
    nc = _get_nc(key, cfg)
    res = run_bass_kernel_spmd(
        nc, in_maps, core_ids=list(range(BATCH)), trace=trace, **kw
    )
    outs = np.stack([res.results[b]["out"] for b in range(BATCH)])
    return outs.astype(np.float32), res


def kernel(**inputs):
    out, _ = run(inputs)
    return out


if __name__ == "__main__":
    rng = np.random.default_rng(0)
    ins = {
        "normalized_resid_pre": rng.standard_normal((8, SEQ, DM)).astype(np.float32),
        "W_Q": (0.02 * rng.standard_normal((NH, DM, DH))).astype(np.float32),
        "b_Q": np.zeros((NH, DH), np.float32),
        "W_K": (0.02 * rng.standard_normal((NH, DM, DH))).astype(np.float32),
        "b_K": np.zeros((NH, DH), np.float32),
        "W_V": (0.02 * rng.standard_normal((NH, DM, DH))).astype(np.float32),
        "b_V": np.zeros((NH, DH), np.float32),
        "W_O": (0.02 * rng.standard_normal((NH, DH, DM))).astype(np.float32),
        "b_O": np.zeros((DM,), np.float32),
    }
    out = kernel(**ins)
    print("kernel output", out.shape, out.dtype, float(np.abs(out).max()))



# revision 3
# speedup vs baseline: 1.0761x; 1.0761x over previous
"""Causal multi-head attention on 8 Trainium2 NeuronCores.

Problem: nn_Attention_46643344835180
  x: [8, 1024, 768], 12 heads x 64 dh, causal softmax attention + output proj.

Sharding: data-parallel over batch (8 batch elements -> 8 cores, no collectives).

v2: full bf16 compute (PSUM accumulation stays f32), host-side transpose of x
(xT fed directly, no PE transposes), all weights resident in SBUF as row
tiles, ragged exp, psum-pool cycling for the output projection.

Per-core dataflow (batch element b):
  xT = x_b.T (host)                                                  [768, 1024]
  QT = Wq.T @ xT  (+bq)            heads stacked on partitions       [768, 1024]
  KT = Wk.T @ xT  (+bk)                                              [768, 1024]
  V  = x_b @ Wv   (+bv)            + interleaved ones column         [1024, 12*66]
  per head h, query-chunk qc (512):
    S^T[k,q] = KT_h.T @ QT_h          keys on partitions
    P^T = exp(S^T / 8)                ScalarE, batched over 2 key-blocks
    causal: one 128-wide-mask multiply per diagonal block
    z^T[65,512] += [V_h | 1].T @ P^T  row 64 accumulates the denominator
    ZT_h = z^T[0:64] * approx(1/z^T[64])   (denom -> reciprocal ->
           gpsimd partition_broadcast -> multiply)
  out = ZT.T @ Wo (+bo)                                              [1024, 768]
"""

import sys

sys.path.insert(0, "/opt/trn_rl_repo")

import ml_dtypes
import numpy as np

import concourse.bass as bass
import concourse.mybir as mybir
import concourse.tile as tile
from concourse import bacc
from concourse.bass_utils import run_bass_kernel_spmd

F32 = mybir.dt.float32
BF16 = mybir.dt.bfloat16
AF = mybir.ActivationFunctionType

SEQ = 1024
DM = 768
NH = 12
DH = 64
VH = DH + 2  # V head stride: 64 dims + ones col + pad (keeps 4B alignment)
BATCH = 8
NQT = SEQ // 128  # 8 seq tiles of 128
NDT = DM // 128  # 6 d_model tiles
QC = 512  # query chunk (moving dim)
NQC = SEQ // QC  # 2
WARMUP = 40  # HAM warmup matmuls (bf16 N=128, ~110ns each cold)


def build(with_bq, with_bk, with_bv, with_bo, debug_taps=False):
    nc = bacc.Bacc("TRN2", target_bir_lowering=False, debug=False)

    xt = nc.dram_tensor("xt", [DM, SEQ], BF16, kind="ExternalInput")
    wq = nc.dram_tensor("wq", [DM, DM], BF16, kind="ExternalInput")
    wk = nc.dram_tensor("wk", [DM, DM], BF16, kind="ExternalInput")
    wv = nc.dram_tensor("wv", [DM, DM], BF16, kind="ExternalInput")
    wo = nc.dram_tensor("wo", [DM, DM], BF16, kind="ExternalInput")
    wmask = nc.dram_tensor("wmask", [128, 128], BF16, kind="ExternalInput")
    identin = nc.dram_tensor("identin", [128, 128], BF16, kind="ExternalInput")
    onesc = nc.dram_tensor("onesc", [128, NH], BF16, kind="ExternalInput")
    bq = bk = bv = bo = None
    if with_bq:
        bq = nc.dram_tensor("bq", [128, NDT], F32, kind="ExternalInput")
    if with_bk:
        bk = nc.dram_tensor("bk", [128, NDT], F32, kind="ExternalInput")
    if with_bv:
        bv = nc.dram_tensor("bv", [1, DM], F32, kind="ExternalInput")
    if with_bo:
        bo = nc.dram_tensor("bo", [1, DM], F32, kind="ExternalInput")
    out = nc.dram_tensor("out", [SEQ, DM], F32, kind="ExternalOutput")
    taps = {}
    if debug_taps:
        for nm in ("QT_d", "KT_d", "ZT_d"):
            taps[nm] = nc.dram_tensor(nm, [DM, SEQ], BF16, kind="ExternalOutput")
        taps["V_d"] = nc.dram_tensor(
            "V_d", [SEQ, NH * VH], BF16, kind="ExternalOutput"
        )

    with tile.TileContext(nc) as tc:
        with (
            tc.tile_pool(name="persist", bufs=1) as persist,
            tc.tile_pool(name="pt", bufs=4) as pt_pool,
            tc.tile_pool(name="small", bufs=2) as small,
            tc.tile_pool(name="outst", bufs=2) as out_pool,
            tc.tile_pool(name="ps_st", bufs=2, space="PSUM") as ps_st,
            tc.tile_pool(name="ps_z", bufs=3, space="PSUM") as ps_z,
            tc.tile_pool(name="ps_mm", bufs=1, space="PSUM") as ps_mm,
        ):
            # ---- warmup constant first, then activations/weights ----
            ident = persist.tile([128, 128], BF16, tag="ident", name="ident")
            nc.sync.dma_start(out=ident, in_=identin[:, :])
            # HAM warmup: dummy matmuls while the input DMAs land, so the
            # projections start at 2.4GHz instead of the cold 1.2GHz
            warm_ps = ps_mm.tile(
                [128, 128], F32, tag="proj", name="warm", padded_shape=[128, QC]
            )
            for _ in range(WARMUP):
                nc.tensor.matmul(warm_ps, lhsT=ident, rhs=ident, start=True, stop=True)

            xT = [
                persist.tile([128, SEQ], BF16, tag=f"xT{d}", name=f"xT{d}")
                for d in range(NDT)
            ]
            for d in range(NDT):
                nc.sync.dma_start(out=xT[d], in_=xt[d * 128 : (d + 1) * 128, :])

            # wq/wk row tiles, split into column halves so head-pair 0's
            # slices land before the full weight load completes
            WQ = [
                persist.tile([128, DM], BF16, tag=f"WQ{d}", name=f"WQ{d}")
                for d in range(NDT)
            ]
            WK = [
                persist.tile([128, DM], BF16, tag=f"WK{d}", name=f"WK{d}")
                for d in range(NDT)
            ]
            for lo, hi in ((0, 256), (256, DM)):
                for src, dst in ((wq, WQ), (wk, WK)):
                    for d in range(NDT):
                        nc.sync.dma_start(
                            out=dst[d][:, lo:hi],
                            in_=src[d * 128 : (d + 1) * 128, lo:hi],
                        )
            WV = [
                persist.tile([128, DM], BF16, tag=f"WV{d}", name=f"WV{d}")
                for d in range(NDT)
            ]
            for d in range(NDT):
                nc.sync.dma_start(out=WV[d], in_=wv[d * 128 : (d + 1) * 128, :])

            wm_t = persist.tile([128, 128], BF16, tag="wmask", name="wmask")
            nc.sync.dma_start(out=wm_t, in_=wmask[:, :])

            bias_tiles = {}
            if with_bq:
                t = persist.tile([128, NDT], F32, tag="bq", name="bq")
                nc.sync.dma_start(out=t, in_=bq[:, :])
                bias_tiles["bq"] = t
            if with_bk:
                t = persist.tile([128, NDT], F32, tag="bk", name="bk")
                nc.sync.dma_start(out=t, in_=bk[:, :])
                bias_tiles["bk"] = t
            if with_bv:
                t = persist.tile([128, DM], F32, tag="bv", name="bv")
                nc.sync.dma_start(out=t, in_=bv[0:1, :].to_broadcast((128, DM)))
                bias_tiles["bv"] = t
            if with_bo:
                t = persist.tile([128, DM], F32, tag="bo", name="bo")
                nc.sync.dma_start(out=t, in_=bo[0:1, :].to_broadcast((128, DM)))
                bias_tiles["bo"] = t

            # ---- persistent activations ----
            QT = [
                persist.tile([128, SEQ], BF16, tag=f"QT{d}", name=f"QT{d}")
                for d in range(NDT)
            ]
            KT = [
                persist.tile([128, SEQ], BF16, tag=f"KT{d}", name=f"KT{d}")
                for d in range(NDT)
            ]
            V = [
                persist.tile([128, NH * VH], BF16, tag=f"V{s}", name=f"V{s}")
                for s in range(NQT)
            ]
            for s in range(NQT):
                vv = V[s].rearrange("p (h e) -> p h e", e=VH)
                nc.sync.dma_start(
                    out=vv[:, :, DH : DH + 1],
                    in_=onesc[:, :].rearrange("p (h o) -> p h o", o=1),
                )
            ZT = [
                persist.tile([128, SEQ], BF16, tag=f"ZT{d}", name=f"ZT{d}")
                for d in range(NDT)
            ]

            def qk_proj(hp):
                # project QT/KT tile hp (heads 2hp, 2hp+1)
                for W, dst, bkey in ((WQ, QT, "bq"), (WK, KT, "bk")):
                    for c in range(NQC):
                        acc = ps_mm.tile([128, QC], F32, tag="proj", name="proj")
                        for d in range(NDT):
                            nc.tensor.matmul(
                                acc,
                                lhsT=W[d][:, hp * 128 : (hp + 1) * 128],
                                rhs=xT[d][:, c * QC : (c + 1) * QC],
                                start=(d == 0),
                                stop=(d == NDT - 1),
                            )
                        o = dst[hp][:, c * QC : (c + 1) * QC]
                        if bkey in bias_tiles:
                            nc.vector.tensor_scalar_add(
                                o, acc, bias_tiles[bkey][:, hp : hp + 1]
                            )
                        else:
                            nc.vector.tensor_copy(o, acc)

            NVC = 2
            VC = DM // NVC  # 384

            def v_proj(s):
                for c in range(NVC):
                    acc = ps_st.tile(
                        [128, VC], F32, tag="st", name="vacc",
                        padded_shape=[128, 2 * QC],
                    )
                    for d in range(NDT):
                        nc.tensor.matmul(
                            acc,
                            lhsT=xT[d][:, s * 128 : (s + 1) * 128],
                            rhs=WV[d][:, c * VC : (c + 1) * VC],
                            start=(d == 0),
                            stop=(d == NDT - 1),
                        )
                    nh2 = VC // DH  # heads per chunk (6)
                    o = V[s].rearrange("p (h e) -> p h e", e=VH)[
                        :, c * nh2 : (c + 1) * nh2, 0:DH
                    ]
                    if "bv" in bias_tiles:
                        nc.vector.tensor_add(
                            o,
                            acc.rearrange("p (h e) -> p h e", e=DH),
                            bias_tiles["bv"][:, c * VC : (c + 1) * VC].rearrange(
                                "p (h e) -> p h e", e=DH
                            ),
                        )
                    else:
                        nc.vector.tensor_copy(
                            o, acc.rearrange("p (h e) -> p h e", e=DH)
                        )

            # ---- attention unit: head pair hp, query chunk c ----
            def attn_unit(hp, c):
                zps = {}
                for px in (0, 64):  # head A in partitions 0:64, B in 64:128
                    zps[px] = ps_z.tile([128, QC], F32, tag="z", name="z")
                nkb = 4 * (c + 1)  # causal: key blocks 0..nkb-1
                for g in range(0, nkb, 2):  # groups of 2 key-blocks
                    gsz = min(2, nkb - g)
                    # columns [0:doff) of a diagonal block are fully causal-
                    # masked: skip them in scores, exp and PV (ragged-N)
                    doffs = [max(0, (g + j) * 128 - c * QC) for j in range(gsz)]
                    sts = {}
                    for px in (0, 64):
                        sts[px] = ps_st.tile(
                            [128, gsz * QC], F32, tag="st", name="st"
                        )
                    for j in range(gsz):
                        kb = g + j
                        off = doffs[j]
                        for px in (0, 64):  # adjacent pair -> row-group packed
                            nc.tensor.matmul(
                                sts[px][:, j * QC + off : (j + 1) * QC],
                                lhsT=KT[hp][px : px + 64, kb * 128 : (kb + 1) * 128],
                                rhs=QT[hp][px : px + 64, c * QC + off : (c + 1) * QC],
                                start=True,
                                stop=True,
                            )
                    pts = {}
                    for px in (0, 64):
                        pt = pt_pool.tile([128, 2 * QC], BF16, tag="pt", name="pt")
                        # one exp per group; stale psum in the skipped columns
                        # inside the window is exp'd but never read.
                        nc.scalar.activation(
                            pt[:, doffs[0] : gsz * QC],
                            sts[px][:, doffs[0] : gsz * QC],
                            AF.Exp,
                            scale=0.125,
                        )
                        pts[px] = pt
                    for j in range(gsz):
                        kb = g + j
                        doff = kb * 128 - c * QC
                        off = doffs[j]
                        for px in (0, 64):
                            pt = pts[px]
                            if 0 <= doff < QC:  # diagonal block: 128-wide triangle
                                blk = pt[:, j * QC + doff : j * QC + doff + 128]
                                nc.vector.tensor_mul(blk, blk, wm_t)
                            h = 2 * hp + (1 if px else 0)
                            nc.tensor.matmul(
                                zps[px][0 : DH + 1, off:QC],
                                lhsT=V[kb][:, h * VH : h * VH + DH + 1],
                                rhs=pt[:, j * QC + off : (j + 1) * QC],
                                start=(kb == 0),
                                stop=(kb == nkb - 1),
                            )
                for px in (0, 64):
                    dstage = small.tile([128, QC], F32, tag="dstage", name="dstage")
                    nc.vector.tensor_copy(dstage[0:1, :], zps[px][DH : DH + 1, :])
                    recip = small.tile([128, QC], F32, tag="recip", name="recip")
                    nc.vector.reciprocal_approx_fast(recip, dstage)
                    bcast = small.tile([64, QC], F32, tag="bcast", name="bcast")
                    nc.gpsimd.partition_broadcast(bcast, recip[0:1, :])
                    nc.vector.tensor_mul(
                        ZT[hp][px : px + 64, c * QC : (c + 1) * QC],
                        zps[px][0:64, :],
                        bcast,
                    )

            WO = []

            def o_proj(s):
                ot = out_pool.tile([128, DM], F32, tag="ostage", name="ostage")
                for c in range(NVC):
                    pool, tag = ((ps_mm, "proj"), (ps_z, "z"))[c % 2]
                    acc = pool.tile(
                        [128, VC], F32, tag=tag, name="oacc",
                        padded_shape=[128, QC],
                    )
                    for d in range(NDT):
                        nc.tensor.matmul(
                            acc,
                            lhsT=ZT[d][:, s * 128 : (s + 1) * 128],
                            rhs=WO[d][:, c * VC : (c + 1) * VC],
                            start=(d == 0),
                            stop=(d == NDT - 1),
                        )
                    o = ot[:, c * VC : (c + 1) * VC]
                    if "bo" in bias_tiles:
                        nc.vector.tensor_add(
                            o, acc, bias_tiles["bo"][:, c * VC : (c + 1) * VC]
                        )
                    else:
                        nc.vector.tensor_copy(o, acc)
                nc.sync.dma_start(out=out[s * 128 : (s + 1) * 128, :], in_=ot)

            # ---- schedule ----
            qk_proj(0)
            for s in range(NQT):
                v_proj(s)
            for hp in range(NH // 2):
                if hp + 1 < NH // 2:
                    qk_proj(hp + 1)
                if hp == 4:  # prefetch O-proj weights late in the qc=0 sweep
                    for d in range(NDT):
                        t = persist.tile([128, DM], BF16, tag=f"WO{d}", name=f"WO{d}")
                        nc.sync.dma_start(out=t, in_=wo[d * 128 : (d + 1) * 128, :])
                        WO.append(t)
                attn_unit(hp, 0)
            # interleave qc=1 attention with the first half of the output
            # projection (queries 0..511, whose ZT rows are complete)
            attn_unit(0, 1)
            for hp in range(1, NH // 2):
                o_proj(hp - 1)
                attn_unit(hp, 1)
            o_proj(4)
            for s in range(5, NQT):
                o_proj(s)

            if debug_taps:
                for nm, tiles in (("QT_d", QT), ("KT_d", KT), ("ZT_d", ZT)):
                    for d in range(NDT):
                        nc.sync.dma_start(
                            out=taps[nm][d * 128 : (d + 1) * 128, :],
                            in_=tiles[d][:, :],
                        )
                for s in range(NQT):
                    nc.sync.dma_start(
                        out=taps["V_d"][s * 128 : (s + 1) * 128, :], in_=V[s][:, :]
                    )

    nc.compile()
    return nc


_CACHE = {}


def _get_nc(key):
    if key not in _CACHE:
        _CACHE[key] = build(*key)
    return _CACHE[key]


def _prep(inputs):
    BF = ml_dtypes.bfloat16
    x = np.asarray(inputs["normalized_resid_pre"], np.float32)
    wq = np.ascontiguousarray(
        np.asarray(inputs["W_Q"], np.float32).transpose(1, 0, 2).reshape(DM, DM)
    ).astype(BF)
    wk = np.ascontiguousarray(
        np.asarray(inputs["W_K"], np.float32).transpose(1, 0, 2).reshape(DM, DM)
    ).astype(BF)
    wv = np.ascontiguousarray(
        np.asarray(inputs["W_V"], np.float32).transpose(1, 0, 2).reshape(DM, DM)
    ).astype(BF)
    wo = np.ascontiguousarray(
        np.asarray(inputs["W_O"], np.float32).reshape(DM, DM)
    ).astype(BF)
    bq = np.asarray(inputs["b_Q"], np.float32).reshape(NDT, 128).T
    bk = np.asarray(inputs["b_K"], np.float32).reshape(NDT, 128).T
    bv = np.asarray(inputs["b_V"], np.float32).reshape(1, DM)
    bo = np.asarray(inputs["b_O"], np.float32).reshape(1, DM)
    jj, uu = np.meshgrid(np.arange(128), np.arange(128), indexing="ij")
    wmask = (uu >= jj).astype(BF)
    onesc = np.ones((128, NH), BF)
    key = (
        bool(np.any(bq)),
        bool(np.any(bk)),
        bool(np.any(bv)),
        bool(np.any(bo)),
    )
    common = {
        "wq": wq, "wk": wk, "wv": wv, "wo": wo, "wmask": wmask, "onesc": onesc,
        "identin": np.eye(128, dtype=np.float32).astype(BF),
    }
    if key[0]:
        common["bq"] = np.ascontiguousarray(bq)
    if key[1]:
        common["bk"] = np.ascontiguousarray(bk)
    if key[2]:
        common["bv"] = np.ascontiguousarray(bv)
    if key[3]:
        common["bo"] = np.ascontiguousarray(bo)
    in_maps = [
        dict(common, xt=np.ascontiguousarray(x[b].T).astype(BF))
        for b in range(BATCH)
    ]
    return key, in_maps


def run(inputs, trace=False, **kw):
    key, in_maps = _prep(inputs)
    nc = _get_nc(key)
    res = run_bass_kernel_spmd(
        nc, in_maps, core_ids=list(range(BATCH)), trace=trace, **kw
    )
    outs = np.stack([res.results[b]["out"] for b in range(BATCH)])
    return outs.astype(np.float32), res


def kernel(**inputs):
    out, _ = run(inputs)
    return out


if __name__ == "__main__":
    rng = np.random.default_rng(0)
    ins = {
        "normalized_resid_pre": rng.standard_normal((8, SEQ, DM)).astype(np.float32),
        "W_Q": (0.02 * rng.standard_normal((NH, DM, DH))).astype(np.float32),
        "b_Q": np.zeros((NH, DH), np.float32),
        "W_K": (0.02 * rng.standard_normal((NH, DM, DH))).astype(np.float32),
        "b_K": np.zeros((NH, DH), np.float32),
        "W_V": (0.02 * rng.standard_normal((NH, DM, DH))).astype(np.float32),
        "b_V": np.zeros((NH, DH), np.float32),
        "W_O": (0.02 * rng.standard_normal((NH, DH, DM))).astype(np.float32),
        "b_O": np.zeros((DM,), np.float32),
    }
    out = kernel(**ins)
    print("kernel output", out.shape, out.dtype, float(np.abs(out).max()))


# revision 6
# speedup vs baseline: 1.2535x; 1.1649x over previous
"""Causal multi-head attention on 8 Trainium2 NeuronCores.

Problem: nn_Attention_46643344835180
  x: [8, 1024, 768], 12 heads x 64 dh, causal softmax attention + output proj.

Sharding: data-parallel over batch (8 batch elements -> 8 cores, no collectives).

v3: full bf16 compute (PSUM stays f32), host-side transpose of x (xT fed
directly), weights resident in SBUF, dual DMA rings (sync + scalar HWDGE),
QK/V psum->sbuf copies on the Scalar engine (idle during the projection
phase), and a globally software-pipelined attention sweep: scores of group
g+1 are issued before exp/PV of group g, with projection/output chunks
spread between groups as PE filler.

Per-core dataflow (batch element b):
  xT = x_b.T (host)                                                  [768, 1024]
  QT = Wq.T @ xT  (+bq)            heads stacked on partitions       [768, 1024]
  KT = Wk.T @ xT  (+bk)                                              [768, 1024]
  V  = x_b @ Wv   (+bv)            + interleaved ones column         [1024, 12*66]
  per head h, query-chunk qc (512):
    S^T[k,q] = KT_h.T @ QT_h          keys on partitions
    P^T = exp(S^T / 8)                ScalarE, batched over 2 key-blocks
    causal: one 128-wide-mask multiply per diagonal block
    z^T[65,512] += [V_h | 1].T @ P^T  row 64 accumulates the denominator
    ZT_h = z^T[0:64] * approx(1/z^T[64])   (denom -> reciprocal ->
           gpsimd partition_broadcast -> multiply)
  out = ZT.T @ Wo (+bo)                                              [1024, 768]
"""

import sys

sys.path.insert(0, "/opt/trn_rl_repo")

from collections import deque

import ml_dtypes
import numpy as np

import concourse.bass as bass
import concourse.mybir as mybir
import concourse.tile as tile
from concourse import bacc
from concourse.bass_utils import run_bass_kernel_spmd

F32 = mybir.dt.float32
BF16 = mybir.dt.bfloat16
AF = mybir.ActivationFunctionType

SEQ = 1024
DM = 768
NH = 12
DH = 64
VH = DH + 2  # V head stride: 64 dims + ones col + pad (keeps 4B alignment)
BATCH = 8
NQT = SEQ // 128  # 8 seq tiles of 128
NDT = DM // 128  # 6 d_model tiles
QC = 512  # query chunk (moving dim)
NQC = SEQ // QC  # 2
WARMUP = 64  # HAM warmup matmuls (bf16 N=128, ~110ns each cold)


def build(with_bq, with_bk, with_bv, with_bo, debug_taps=False):
    nc = bacc.Bacc("TRN2", target_bir_lowering=False, debug=False)

    xt = nc.dram_tensor("xt", [DM, SEQ], BF16, kind="ExternalInput")
    wq = nc.dram_tensor("wq", [DM, DM], BF16, kind="ExternalInput")
    wk = nc.dram_tensor("wk", [DM, DM], BF16, kind="ExternalInput")
    wv = nc.dram_tensor("wv", [DM, DM], BF16, kind="ExternalInput")
    wo = nc.dram_tensor("wo", [DM, DM], BF16, kind="ExternalInput")
    wmask = nc.dram_tensor("wmask", [128, 128], BF16, kind="ExternalInput")
    identin = nc.dram_tensor("identin", [128, 128], BF16, kind="ExternalInput")
    onesc = nc.dram_tensor("onesc", [128, NH], BF16, kind="ExternalInput")
    bq = bk = bv = bo = None
    if with_bq:
        bq = nc.dram_tensor("bq", [128, NDT], F32, kind="ExternalInput")
    if with_bk:
        bk = nc.dram_tensor("bk", [128, NDT], F32, kind="ExternalInput")
    if with_bv:
        bv = nc.dram_tensor("bv", [1, DM], F32, kind="ExternalInput")
    if with_bo:
        bo = nc.dram_tensor("bo", [1, DM], F32, kind="ExternalInput")
    out = nc.dram_tensor("out", [SEQ, DM], F32, kind="ExternalOutput")
    taps = {}
    if debug_taps:
        for nm in ("QT_d", "KT_d", "ZT_d"):
            taps[nm] = nc.dram_tensor(nm, [DM, SEQ], BF16, kind="ExternalOutput")
        taps["V_d"] = nc.dram_tensor(
            "V_d", [SEQ, NH * VH], BF16, kind="ExternalOutput"
        )

    with tile.TileContext(nc) as tc:
        with (
            tc.tile_pool(name="persist", bufs=1) as persist,
            tc.tile_pool(name="pt", bufs=4) as pt_pool,
            tc.tile_pool(name="small", bufs=2) as small,
            tc.tile_pool(name="outst", bufs=2) as out_pool,
            tc.tile_pool(name="ps_st", bufs=2, space="PSUM") as ps_st,
            tc.tile_pool(name="ps_z", bufs=3, space="PSUM") as ps_z,
            tc.tile_pool(name="ps_mm", bufs=1, space="PSUM") as ps_mm,
        ):
            # ---- sync ring: ident, xT, WV, mask/ones.  scalar ring: wq/wk
            # (column-split so head pairs 0-1 unblock early), wo later ----
            ident = persist.tile([128, 128], BF16, tag="ident", name="ident")
            nc.sync.dma_start(out=ident, in_=identin[:, :])
            warm_ps = ps_mm.tile(
                [128, 128], F32, tag="proj", name="warm", padded_shape=[128, QC]
            )
            for _ in range(WARMUP):
                nc.tensor.matmul(warm_ps, lhsT=ident, rhs=ident, start=True, stop=True)

            # xt split across both rings so the projections unblock early
            xT = [
                persist.tile([128, SEQ], BF16, tag=f"xT{d}", name=f"xT{d}")
                for d in range(NDT)
            ]
            for d in range(NDT):
                eng = nc.sync if d < 3 else nc.scalar
                eng.dma_start(out=xT[d], in_=xt[d * 128 : (d + 1) * 128, :])

            WQ = [
                persist.tile([128, DM], BF16, tag=f"WQ{d}", name=f"WQ{d}")
                for d in range(NDT)
            ]
            WK = [
                persist.tile([128, DM], BF16, tag=f"WK{d}", name=f"WK{d}")
                for d in range(NDT)
            ]
            WV = [
                persist.tile([128, DM], BF16, tag=f"WV{d}", name=f"WV{d}")
                for d in range(NDT)
            ]
            for d in range(NDT):
                nc.sync.dma_start(out=WV[d], in_=wv[d * 128 : (d + 1) * 128, :])
            for lo, hi in ((0, 256), (256, DM)):
                for src, dst in ((wq, WQ), (wk, WK)):
                    for d in range(NDT):
                        nc.scalar.dma_start(
                            out=dst[d][:, lo:hi],
                            in_=src[d * 128 : (d + 1) * 128, lo:hi],
                        )

            wm_t = persist.tile([128, 128], BF16, tag="wmask", name="wmask")
            nc.sync.dma_start(out=wm_t, in_=wmask[:, :])

            bias_tiles = {}
            if with_bq:
                t = persist.tile([128, NDT], F32, tag="bq", name="bq")
                nc.scalar.dma_start(out=t, in_=bq[:, :])
                bias_tiles["bq"] = t
            if with_bk:
                t = persist.tile([128, NDT], F32, tag="bk", name="bk")
                nc.scalar.dma_start(out=t, in_=bk[:, :])
                bias_tiles["bk"] = t
            if with_bv:
                t = persist.tile([128, DM], F32, tag="bv", name="bv")
                nc.scalar.dma_start(out=t, in_=bv[0:1, :].to_broadcast((128, DM)))
                bias_tiles["bv"] = t
            if with_bo:
                t = persist.tile([128, DM], F32, tag="bo", name="bo")
                nc.scalar.dma_start(out=t, in_=bo[0:1, :].to_broadcast((128, DM)))
                bias_tiles["bo"] = t

            QT = [
                persist.tile([128, SEQ], BF16, tag=f"QT{d}", name=f"QT{d}")
                for d in range(NDT)
            ]
            KT = [
                persist.tile([128, SEQ], BF16, tag=f"KT{d}", name=f"KT{d}")
                for d in range(NDT)
            ]
            V = [
                persist.tile([128, NH * VH], BF16, tag=f"V{s}", name=f"V{s}")
                for s in range(NQT)
            ]
            for s in range(NQT):
                vv = V[s].rearrange("p (h e) -> p h e", e=VH)
                nc.sync.dma_start(
                    out=vv[:, :, DH : DH + 1],
                    in_=onesc[:, :].rearrange("p (h o) -> p h o", o=1),
                )
            ZT = [
                persist.tile([128, SEQ], BF16, tag=f"ZT{d}", name=f"ZT{d}")
                for d in range(NDT)
            ]

            # ---- projection chunks (each = one psum round trip) ----
            def qk_chunk(hp, which, c, pool, tag):
                W, dst, bkey = (
                    (WQ, QT, "bq") if which == "q" else (WK, KT, "bk")
                )
                acc = pool.tile(
                    [128, QC], F32, tag=tag, name="proj",
                    padded_shape=[128, 2 * QC] if tag == "st" else [128, QC],
                )
                for d in range(NDT):
                    nc.tensor.matmul(
                        acc,
                        lhsT=W[d][:, hp * 128 : (hp + 1) * 128],
                        rhs=xT[d][:, c * QC : (c + 1) * QC],
                        start=(d == 0),
                        stop=(d == NDT - 1),
                    )
                o = dst[hp][:, c * QC : (c + 1) * QC]
                if bkey in bias_tiles:
                    nc.vector.tensor_scalar_add(
                        o, acc, bias_tiles[bkey][:, hp : hp + 1]
                    )
                else:
                    nc.scalar.activation(o, acc, AF.Copy)

            def qk_chunks(hp):
                return [
                    (lambda which=which, c=c: qk_chunk(hp, which, c, ps_mm, "proj"))
                    for which in ("q", "k")
                    for c in range(NQC)
                ]

            NVC = 2
            VC = DM // NVC  # 384

            def v_chunk(s, c):
                acc = ps_st.tile(
                    [128, VC], F32, tag="st", name="vacc",
                    padded_shape=[128, 2 * QC],
                )
                for d in range(NDT):
                    nc.tensor.matmul(
                        acc,
                        lhsT=xT[d][:, s * 128 : (s + 1) * 128],
                        rhs=WV[d][:, c * VC : (c + 1) * VC],
                        start=(d == 0),
                        stop=(d == NDT - 1),
                    )
                nh2 = VC // DH  # heads per chunk (6)
                o = V[s].rearrange("p (h e) -> p h e", e=VH)[
                    :, c * nh2 : (c + 1) * nh2, 0:DH
                ]
                if "bv" in bias_tiles:
                    nc.vector.tensor_add(
                        o,
                        acc.rearrange("p (h e) -> p h e", e=DH),
                        bias_tiles["bv"][:, c * VC : (c + 1) * VC].rearrange(
                            "p (h e) -> p h e", e=DH
                        ),
                    )
                else:
                    nc.scalar.activation(
                        o, acc.rearrange("p (h e) -> p h e", e=DH), AF.Copy
                    )

            def v_chunks(s):
                return [lambda c=c: v_chunk(s, c) for c in range(NVC)]

            WO = []

            def wo_load():
                # sync ring: idle mid-attention (scalar ring would block ACT)
                for d in range(NDT):
                    t = persist.tile([128, DM], BF16, tag=f"WO{d}", name=f"WO{d}")
                    nc.sync.dma_start(out=t, in_=wo[d * 128 : (d + 1) * 128, :])
                    WO.append(t)

            def o_chunks(s):
                ot = [None]

                def chunk(c):
                    if c == 0:
                        ot[0] = out_pool.tile([128, DM], F32, tag="ostage", name="ostage")
                    pool, tag = ((ps_mm, "proj"), (ps_z, "z"))[c % 2]
                    acc = pool.tile(
                        [128, VC], F32, tag=tag, name="oacc",
                        padded_shape=[128, QC],
                    )
                    for d in range(NDT):
                        nc.tensor.matmul(
                            acc,
                            lhsT=ZT[d][:, s * 128 : (s + 1) * 128],
                            rhs=WO[d][:, c * VC : (c + 1) * VC],
                            start=(d == 0),
                            stop=(d == NDT - 1),
                        )
                    o = ot[0][:, c * VC : (c + 1) * VC]
                    if "bo" in bias_tiles:
                        nc.vector.tensor_add(
                            o, acc, bias_tiles["bo"][:, c * VC : (c + 1) * VC]
                        )
                    else:
                        nc.vector.tensor_copy(o, acc)
                    if c == NVC - 1:
                        nc.sync.dma_start(
                            out=out[s * 128 : (s + 1) * 128, :], in_=ot[0]
                        )

                return [lambda c=c: chunk(c) for c in range(NVC)]

            # ---- pipelined attention sweep ----
            zps_of = {}

            def issue_scores(hp, c, g, gsz):
                doffs = [max(0, (g + j) * 128 - c * QC) for j in range(gsz)]
                sts = {}
                for px in (0, 64):
                    sts[px] = ps_st.tile([128, gsz * QC], F32, tag="st", name="st")
                for j in range(gsz):
                    kb = g + j
                    off = doffs[j]
                    for px in (0, 64):
                        nc.tensor.matmul(
                            sts[px][:, j * QC + off : (j + 1) * QC],
                            lhsT=KT[hp][px : px + 64, kb * 128 : (kb + 1) * 128],
                            rhs=QT[hp][px : px + 64, c * QC + off : (c + 1) * QC],
                            start=True,
                            stop=True,
                        )
                return sts, doffs

            def issue_expv(hp, c, g, gsz, sts, doffs, last):
                nkb = 4 * (c + 1)
                if g == 0:
                    zps_of[(hp, c)] = {
                        px: ps_z.tile([128, QC], F32, tag="z", name="z")
                        for px in (0, 64)
                    }
                zps = zps_of[(hp, c)]
                pts = {}
                for px in (0, 64):
                    pt = pt_pool.tile([128, 2 * QC], BF16, tag="pt", name="pt")
                    nc.scalar.activation(
                        pt[:, doffs[0] : gsz * QC],
                        sts[px][:, doffs[0] : gsz * QC],
                        AF.Exp,
                        scale=0.125,
                    )
                    pts[px] = pt
                for j in range(gsz):
                    kb = g + j
                    doff = kb * 128 - c * QC
                    off = doffs[j]
                    for px in (0, 64):
                        pt = pts[px]
                        if 0 <= doff < QC:  # diagonal block: 128-wide triangle
                            blk = pt[:, j * QC + doff : j * QC + doff + 128]
                            nc.vector.tensor_mul(blk, blk, wm_t)
                        h = 2 * hp + (1 if px else 0)
                        nc.tensor.matmul(
                            zps[px][0 : DH + 1, off:QC],
                            lhsT=V[kb][:, h * VH : h * VH + DH + 1],
                            rhs=pt[:, j * QC + off : (j + 1) * QC],
                            start=(kb == 0),
                            stop=(kb == nkb - 1),
                        )
                if last:
                    for px in (0, 64):
                        dstage = small.tile([128, QC], F32, tag="dstage", name="dstage")
                        nc.vector.tensor_copy(dstage[0:1, :], zps[px][DH : DH + 1, :])
                        recip = small.tile([128, QC], F32, tag="recip", name="recip")
                        nc.vector.reciprocal_approx_fast(recip, dstage)
                        bcast = small.tile([64, QC], F32, tag="bcast", name="bcast")
                        nc.gpsimd.partition_broadcast(bcast, recip[0:1, :])
                        nc.vector.tensor_mul(
                            ZT[hp][px : px + 64, c * QC : (c + 1) * QC],
                            zps[px][0:64, :],
                            bcast,
                        )
                    del zps_of[(hp, c)]

            # ---- pre-phase: warmup already issued; project heads 0-3 (hp
            # 0,1) alternating psum rings, then V tiles 0-3 ----
            for hp in (0, 1):
                for i, (which, c) in enumerate(
                    (w, c) for w in ("q", "k") for c in range(NQC)
                ):
                    pool, tag = ((ps_mm, "proj"), (ps_st, "st"))[i % 2]
                    qk_chunk(hp, which, c, pool, tag)
            for s in range(4):
                for f in v_chunks(s):
                    f()

            units = [(hp, 0) for hp in range(NH // 2)] + [
                (hp, 1) for hp in range(NH // 2)
            ]
            fillers = {
                0: v_chunks(4) + v_chunks(5),
                1: qk_chunks(2) + v_chunks(6),
                2: qk_chunks(3) + v_chunks(7),
                3: qk_chunks(4),
                4: qk_chunks(5) + [wo_load],
                7: o_chunks(0),
                8: o_chunks(1),
                9: o_chunks(2),
                10: o_chunks(3),
            }

            pending = [None]

            def flush():
                if pending[0] is not None:
                    fn = pending[0]
                    pending[0] = None
                    fn()

            for ui, (hp, c) in enumerate(units):
                nkb = 4 * (c + 1)
                glist = [(g, min(2, nkb - g)) for g in range(0, nkb, 2)]
                chunks = deque(fillers.get(ui, []))
                n = len(glist)
                for gi, (g, gsz) in enumerate(glist):
                    sts, doffs = issue_scores(hp, c, g, gsz)
                    k = -(-len(chunks) // (n - gi)) if chunks else 0
                    for i in range(k):
                        chunks.popleft()()
                        if i == 0:
                            flush()
                    if k == 0:
                        flush()
                    pending[0] = (
                        lambda hp=hp, c=c, g=g, gsz=gsz, sts=sts, doffs=doffs,
                        last=(gi == n - 1): issue_expv(hp, c, g, gsz, sts, doffs, last)
                    )
            flush()

            # ---- tail: output rows 512-1024 ----
            for s in range(4, NQT):
                for f in o_chunks(s):
                    f()

            if debug_taps:
                for nm, tiles in (("QT_d", QT), ("KT_d", KT), ("ZT_d", ZT)):
                    for d in range(NDT):
                        nc.sync.dma_start(
                            out=taps[nm][d * 128 : (d + 1) * 128, :],
                            in_=tiles[d][:, :],
                        )
                for s in range(NQT):
                    nc.sync.dma_start(
                        out=taps["V_d"][s * 128 : (s + 1) * 128, :], in_=V[s][:, :]
                    )

    nc.compile()
    return nc


_CACHE = {}


def _get_nc(key):
    if key not in _CACHE:
        _CACHE[key] = build(*key)
    return _CACHE[key]


def _prep(inputs):
    BF = ml_dtypes.bfloat16
    x = np.asarray(inputs["normalized_resid_pre"], np.float32)
    wq = np.ascontiguousarray(
        np.asarray(inputs["W_Q"], np.float32).transpose(1, 0, 2).reshape(DM, DM)
    ).astype(BF)
    wk = np.ascontiguousarray(
        np.asarray(inputs["W_K"], np.float32).transpose(1, 0, 2).reshape(DM, DM)
    ).astype(BF)
    wv = np.ascontiguousarray(
        np.asarray(inputs["W_V"], np.float32).transpose(1, 0, 2).reshape(DM, DM)
    ).astype(BF)
    wo = np.ascontiguousarray(
        np.asarray(inputs["W_O"], np.float32).reshape(DM, DM)
    ).astype(BF)
    bq = np.asarray(inputs["b_Q"], np.float32).reshape(NDT, 128).T
    bk = np.asarray(inputs["b_K"], np.float32).reshape(NDT, 128).T
    bv = np.asarray(inputs["b_V"], np.float32).reshape(1, DM)
    bo = np.asarray(inputs["b_O"], np.float32).reshape(1, DM)
    jj, uu = np.meshgrid(np.arange(128), np.arange(128), indexing="ij")
    wmask = (uu >= jj).astype(BF)
    onesc = np.ones((128, NH), BF)
    key = (
        bool(np.any(bq)),
        bool(np.any(bk)),
        bool(np.any(bv)),
        bool(np.any(bo)),
    )
    common = {
        "wq": wq, "wk": wk, "wv": wv, "wo": wo, "wmask": wmask, "onesc": onesc,
        "identin": np.eye(128, dtype=np.float32).astype(BF),
    }
    if key[0]:
        common["bq"] = np.ascontiguousarray(bq)
    if key[1]:
        common["bk"] = np.ascontiguousarray(bk)
    if key[2]:
        common["bv"] = np.ascontiguousarray(bv)
    if key[3]:
        common["bo"] = np.ascontiguousarray(bo)
    in_maps = [
        dict(common, xt=np.ascontiguousarray(x[b].T).astype(BF))
        for b in range(BATCH)
    ]
    return key, in_maps


def run(inputs, trace=False, **kw):
    key, in_maps = _prep(inputs)
    nc = _get_nc(key)
    res = run_bass_kernel_spmd(
        nc, in_maps, core_ids=list(range(BATCH)), trace=trace, **kw
    )
    outs = np.stack([res.results[b]["out"] for b in range(BATCH)])
    return outs.astype(np.float32), res


def kernel(**inputs):
    out, _ = run(inputs)
    return out


if __name__ == "__main__":
    rng = np.random.default_rng(0)
    ins = {
        "normalized_resid_pre": rng.standard_normal((8, SEQ, DM)).astype(np.float32),
        "W_Q": (0.02 * rng.standard_normal((NH, DM, DH))).astype(np.float32),
        "b_Q": np.zeros((NH, DH), np.float32),
        "W_K": (0.02 * rng.standard_normal((NH, DM, DH))).astype(np.float32),
        "b_K": np.zeros((NH, DH), np.float32),
        "W_V": (0.02 * rng.standard_normal((NH, DM, DH))).astype(np.float32),
        "b_V": np.zeros((NH, DH), np.float32),
        "W_O": (0.02 * rng.standard_normal((NH, DH, DM))).astype(np.float32),
        "b_O": np.zeros((DM,), np.float32),
    }
    out = kernel(**ins)
    print("kernel output", out.shape, out.dtype, float(np.abs(out).max()))
